# revision 8
# baseline (speedup 1.0000x reference)
"""Trainium2 Bass kernel for BottleNeck attention (8 NeuronCores).

Reference computation (jax, fp32):
    qp = q @ Wq.T + bq          [B=8, L=4096, D=1024]
    kp = k @ Wk.T + bk
    vp = v @ Wv.T + bv
    score = qp[:, :256] @ kp.T / sqrt(D)        [B, 256, L]
    attn  = softmax(score, axis=0)              (softmax over the BATCH axis!)
    out   = attn @ vp                           [B, 256, D]

Strategy:
  * Data-parallel over batch: core c owns batch b=c.
  * Algebraic reassociation avoids projecting full-length k/v:
        qp_T = Wq.T.T @ q.T                     [D, Q]     (per batch)
        qk_T = Wk.T @ qp_T                      [D, Q]
        score_T = k.T.T @ qk_T (+ bias row)     [L, Q]
        E = exp(score_T / 32)
        denom = AllReduce_batch(E)              (axis-0 softmax denominator)
        attn_T = E / denom                      [L, Q]
        av_T = v.T-chunks @ attn_T              [D, Q]
        out = av_T.T @ Wv.T + rowsum(attn) * bv [Q, D]
    This cuts FLOPs ~3x vs projecting kp/vp at full length.
  * Host pre-transposes (k.T, q.T, Wq.T, Wv.T) so no transposes on device.
  * Compute dtype float32r: fp32 bit layout, PE rounds to ~tf32; full PE rate
    at N>=256.

All matmuls contract over the SBUF partition dim; every operand arrives in
its natural (host pre-transposed) layout.
"""

import sys
from contextlib import ExitStack

sys.path.insert(0, "/opt/trn_rl_repo")

import numpy as np

import concourse.bass as bass
import concourse.mybir as mybir
import concourse.tile as tile
from concourse import bacc, bass_utils

B = 8
L = 4096
D = 1024
Q = 256  # bottleneck
N_CORES = 8
P = 128
DC = D // P  # 8 d-chunks
EC = D // P  # 8 e-chunks
LC = L // P  # 32 l-chunks
SCALE = 1.0 / 32.0  # 1/sqrt(1024)

# compute dtype for matmul operands ("bf16" | "fp32r" | "fp32")
COMPUTE = "fp32r"

_CDT = {
    "bf16": mybir.dt.bfloat16,
    "fp32r": mybir.dt.float32r,  # fp32 bits; PE rounds internally (~tf32)
    "fp32": mybir.dt.float32,
}

_cached = {}


def _np_cdt():
    if COMPUTE == "bf16":
        import ml_dtypes

        return np.dtype(ml_dtypes.bfloat16)
    return np.dtype(np.float32)


def build_kernel():
    CDT = _CDT[COMPUTE]
    F32 = mybir.dt.float32

    nc = bacc.Bacc("TRN2", target_bir_lowering=False, debug=False,
                   num_devices=N_CORES)

    # ---- per-core external inputs (host pre-transposed / pre-cast) ----
    kT = nc.dram_tensor("kT", [D, L], CDT, kind="ExternalInput")       # k[b].T
    v_in = nc.dram_tensor("v_in", [L, D], CDT, kind="ExternalInput")   # v[b]
    qT = nc.dram_tensor("qT", [D, Q], CDT, kind="ExternalInput")       # q[b,:Q].T
    wqT = nc.dram_tensor("wqT", [D, D], CDT, kind="ExternalInput")     # Wq.T
    wk = nc.dram_tensor("wk", [D, D], CDT, kind="ExternalInput")       # Wk
    wvT = nc.dram_tensor("wvT", [D, D], CDT, kind="ExternalInput")     # Wv.T
    bq_in = nc.dram_tensor("bq_in", [1, D], CDT, kind="ExternalInput")
    bk_in = nc.dram_tensor("bk_in", [P, EC], CDT, kind="ExternalInput")  # bk.reshape(EC,P).T
    bv_in = nc.dram_tensor("bv_in", [1, D], CDT, kind="ExternalInput")
    ones_r_in = nc.dram_tensor("ones_r", [1, Q], CDT, kind="ExternalInput")
    ones_c_in = nc.dram_tensor("ones_c", [P, 1], CDT, kind="ExternalInput")
    out_ext = nc.dram_tensor("out", [Q, D], F32, kind="ExternalOutput")

    # DRAM views with the partition-chunk structure we DMA through
    kT_v = kT.rearrange("(c p) l -> p c l", p=P)        # [128, 8, 4096]
    wqT_v = wqT.rearrange("(c p) e -> p c e", p=P)      # [128, 8, 1024]
    wk_v = wk.rearrange("(c p) d -> p c d", p=P)
    wvT_v = wvT.rearrange("(c p) e -> p c e", p=P)
    qT_v = qT.rearrange("(c p) q -> p c q", p=P)        # [128, 8, 256]
    v_v = v_in.rearrange("(c p) d -> c p d", p=P)       # [32, 128, 1024]
    out_v = out_ext.rearrange("(m p) e -> p m e", p=P)  # [128, 2, 1024]

    with tile.TileContext(nc) as tc, ExitStack() as top:
        consts = top.enter_context(tc.tile_pool(name="consts", bufs=1))
        qstate = top.enter_context(tc.tile_pool(name="qstate", bufs=1))
        dram = top.enter_context(tc.tile_pool(name="dram", bufs=1, space="DRAM"))

        # ---------------- constants ----------------
        ones_row = consts.tile([1, Q], CDT)       # [1, 256] of 1.0
        ones_col = consts.tile([P, 1], CDT)       # [128, 1] of 1.0
        nc.sync.dma_start(out=ones_row, in_=ones_r_in[:, :])
        nc.sync.dma_start(out=ones_col, in_=ones_c_in[:, :])
        bq_sb = consts.tile([1, D], CDT)
        bk_sb = consts.tile([P, EC], CDT)
        bv_sb = consts.tile([1, D], CDT)
        nc.sync.dma_start(out=bq_sb, in_=bq_in[:, :])
        nc.sync.dma_start(out=bk_sb, in_=bk_in[:, :])
        nc.sync.dma_start(out=bv_sb, in_=bv_in[:, :])

        qpT_sb = qstate.tile([P, EC, Q], CDT)
        qkT_sb = qstate.tile([P, DC, Q], CDT)
        qkb_sb = qstate.tile([1, Q], CDT)
        avT_sb = qstate.tile([P, DC, Q], CDT)
        rs_sb = qstate.tile([1, Q], CDT)

        ps4_ctx = ExitStack()
        ps4 = ps4_ctx.enter_context(
            tc.tile_pool(name="ps4", bufs=4, space="PSUM"))

        # ================ phases A+B: q-side projections ================
        with tc.tile_pool(name="wab", bufs=1) as wab:
            wqT_sb = wab.tile([P, EC, D], CDT)
            wk_sb = wab.tile([P, EC, D], CDT)
            qT_sb = wab.tile([P, DC, Q], CDT)
            nc.sync.dma_start(out=wqT_sb, in_=wqT_v)
            nc.sync.dma_start(out=wk_sb, in_=wk_v)
            nc.sync.dma_start(out=qT_sb, in_=qT_v)

            # qp_T[e,q] = sum_d WqT[d, e-chunk].T @ qT[d, q]  (+ bq x ones)
            for ec in range(EC):
                ps_qp = ps4.tile([P, Q], F32, tag="ps")
                for dc in range(DC):
                    nc.tensor.matmul(
                        ps_qp,
                        wqT_sb[:, dc, ec * P:(ec + 1) * P],
                        qT_sb[:, dc, :],
                        start=(dc == 0), stop=False,
                    )
                nc.tensor.matmul(
                    ps_qp, bq_sb[:, ec * P:(ec + 1) * P], ones_row,
                    start=False, stop=True,
                )
                nc.scalar.copy(qpT_sb[:, ec, :], ps_qp)

            # qk_T[d,q] = sum_e Wk[e, d-chunk].T @ qp_T[e, q]
            for dc in range(DC):
                ps_qk = ps4.tile([P, Q], F32, tag="ps")
                for ec in range(EC):
                    nc.tensor.matmul(
                        ps_qk,
                        wk_sb[:, ec, dc * P:(dc + 1) * P],
                        qpT_sb[:, ec, :],
                        start=(ec == 0), stop=(ec == EC - 1),
                    )
                nc.scalar.copy(qkT_sb[:, dc, :], ps_qk)
            # score bias row: qkb[q] = sum_e bk[e] * qp_T[e, q]
            ps_qkb = ps4.tile([1, Q], F32, tag="ps")
            for ec in range(EC):
                nc.tensor.matmul(
                    ps_qkb, bk_sb[:, ec:ec + 1], qpT_sb[:, ec, :],
                    start=(ec == 0), stop=(ec == EC - 1),
                )
            nc.scalar.copy(qkb_sb, ps_qkb)

        # ================ phase C: score_T -> E -> DRAM ================
        bigctx = ExitStack()
        bigbuf = bigctx.enter_context(tc.tile_pool(name="bigbuf", bufs=1))
        wvp = bigctx.enter_context(tc.tile_pool(name="wvp", bufs=1))
        E_sb = bigbuf.tile([P, LC * Q], F32)          # [128, 8192] 4MB
        wvT_sb = wvp.tile([P, DC, D], CDT)
        nc.sync.dma_start(out=wvT_sb, in_=wvT_v)      # overlaps C on DMA
        E_dram = dram.tile([P, LC * Q], F32)
        denom_dram = dram.tile([P, LC * Q], F32, addr_space="Shared")

        SLAB = 4  # l-chunks per kT slab (512 l positions)
        with tc.tile_pool(name="kslabs", bufs=2) as kslabs:
            for sl in range(LC // SLAB):
                kT_t = kslabs.tile([P, DC, SLAB * P], CDT, tag="kT")
                nc.sync.dma_start(
                    out=kT_t,
                    in_=kT_v[:, :, sl * SLAB * P:(sl + 1) * SLAB * P])
                for s in range(SLAB):
                    lc = sl * SLAB + s
                    ps_s = ps4.tile([P, Q], F32, tag="ps")
                    for dc in range(DC):
                        nc.tensor.matmul(
                            ps_s,
                            kT_t[:, dc, s * P:(s + 1) * P],
                            qkT_sb[:, dc, :],
                            start=(dc == 0), stop=False,
                        )
                    nc.tensor.matmul(
                        ps_s, ones_row[:, :P], qkb_sb,
                        start=False, stop=True,
                    )
                    nc.scalar.activation(
                        out=E_sb[:, lc * Q:(lc + 1) * Q], in_=ps_s,
                        func=mybir.ActivationFunctionType.Exp, scale=SCALE,
                    )
                    nc.sync.dma_start(
                        out=E_dram[:, lc * Q:(lc + 1) * Q],
                        in_=E_sb[:, lc * Q:(lc + 1) * Q],
                    )
        ps4_ctx.close()

        # ================ phase D: AllReduce(denominator) ================
        nc.gpsimd.collective_compute(
            "AllReduce", mybir.AluOpType.add,
            replica_groups=[list(range(N_CORES))],
            ins=[E_dram.opt()], outs=[denom_dram.opt()],
        )

        # ================ phase E: attn = E / denom =====================
        # attn gets its own tile (dtype CDT) so the fp32r matmuls see a
        # properly-rounded producer (the DVE tensor_mul).
        attnp = bigctx.enter_context(tc.tile_pool(name="attnp", bufs=1))
        denom_sb = attnp.tile([P, LC * Q], F32)
        attn_sb = attnp.tile([P, LC * Q], CDT)
        NCH = 8
        CH = LC * Q // NCH
        for i in range(NCH):
            sli = slice(i * CH, (i + 1) * CH)
            nc.sync.dma_start(out=denom_sb[:, sli], in_=denom_dram[:, sli])
            nc.vector.reciprocal(denom_sb[:, sli], denom_sb[:, sli])
            nc.vector.tensor_mul(attn_sb[:, sli], E_sb[:, sli], denom_sb[:, sli])

        # ================ phase F: av_T[d, q] accumulation ===============
        # av_T[d,q] = sum_l v[l, d-chunk].T @ attn_T[l, q]
        with (tc.tile_pool(name="accump", bufs=1, space="PSUM") as accump,
              tc.tile_pool(name="vslabs", bufs=3) as vslabs):
            av_ps = [accump.tile([P, Q], F32, name=f"av_ps_{dc}")
                     for dc in range(DC)]
            for lc in range(LC):
                v_t = vslabs.tile([P, D], CDT, tag="vt")
                nc.sync.dma_start(out=v_t, in_=v_v[lc, :, :])
                at = attn_sb[:, lc * Q:(lc + 1) * Q]
                for dc in range(DC):
                    nc.tensor.matmul(
                        av_ps[dc], v_t[:, dc * P:(dc + 1) * P], at,
                        start=(lc == 0), stop=(lc == LC - 1),
                    )
            for dc in range(DC):
                nc.scalar.copy(avT_sb[:, dc, :], av_ps[dc])

        # ================ phase G: rowsum + out projection ===============
        with (tc.tile_pool(name="outp", bufs=2, space="PSUM") as outp,
              tc.tile_pool(name="rsp", bufs=1, space="PSUM") as rsp,
              tc.tile_pool(name="outsb", bufs=2) as outsb):
            # rowsum(attn)[q] via ones-column matmul
            rs_ps = rsp.tile([1, Q], F32)
            for lc in range(LC):
                nc.tensor.matmul(
                    rs_ps, ones_col, attn_sb[:, lc * Q:(lc + 1) * Q],
                    start=(lc == 0), stop=(lc == LC - 1),
                )
            nc.scalar.copy(rs_sb, rs_ps)

            # out[q,e] = sum_d av_T[d, q-chunk].T @ WvT[d, e] + rs * bv
            NB = D // 512
            for qm in range(Q // P):
                for eb in range(NB):
                    ps_o = outp.tile([P, 512], F32, tag="ps_out")
                    for dc in range(DC):
                        nc.tensor.matmul(
                            ps_o,
                            avT_sb[:, dc, qm * P:(qm + 1) * P],
                            wvT_sb[:, dc, eb * 512:(eb + 1) * 512],
                            start=(dc == 0), stop=False,
                        )
                    nc.tensor.matmul(
                        ps_o,
                        rs_sb[:, qm * P:(qm + 1) * P],
                        bv_sb[:, eb * 512:(eb + 1) * 512],
                        start=False, stop=True,
                    )
                    o_sb = outsb.tile([P, 512], F32, tag="o_sb")
                    nc.vector.tensor_copy(o_sb, ps_o)
                    nc.sync.dma_start(
                        out=out_v[:, qm, eb * 512:(eb + 1) * 512], in_=o_sb)
        bigctx.close()

    nc.compile()
    return nc


def _prep_inputs(q, k, v, Wq, bq, Wk, bk, Wv, bv):
    """Shard + pre-transpose + cast on host. Returns in_maps for 8 cores."""
    cnp = _np_cdt()
    f32 = np.float32

    def c(x):
        return np.ascontiguousarray(np.asarray(x, dtype=f32), dtype=cnp)

    # shared across cores
    wqT = c(np.asarray(Wq, dtype=f32).T)
    wk_ = c(Wk)
    wvT = c(np.asarray(Wv, dtype=f32).T)
    bq_ = c(np.asarray(bq, dtype=f32).reshape(1, D))
    bk_ = c(np.asarray(bk, dtype=f32).reshape(EC, P).T)
    bv_ = c(np.asarray(bv, dtype=f32).reshape(1, D))
    ones_r = np.ones((1, Q), dtype=cnp)
    ones_c = np.ones((P, 1), dtype=cnp)

    in_maps = []
    for b in range(B):
        in_maps.append({
            "kT": c(np.asarray(k[b], dtype=f32).T),
            "v_in": c(v[b]),
            "qT": c(np.asarray(q[b, :Q], dtype=f32).T),
            "wqT": wqT,
            "wk": wk_,
            "wvT": wvT,
            "bq_in": bq_,
            "bk_in": bk_,
            "bv_in": bv_,
            "ones_r": ones_r,
            "ones_c": ones_c,
        })
    return in_maps


def kernel(q, k, v, Wq, bq, Wk, bk, Wv, bv, _trace=False):
    q = np.asarray(q)
    k = np.asarray(k)
    v = np.asarray(v)
    if "nc" not in _cached:
        _cached["nc"] = build_kernel()
    nc = _cached["nc"]
    in_maps = _prep_inputs(q, k, v, Wq, bq, Wk, bk, Wv, bv)
    res = bass_utils.run_bass_kernel_spmd(
        nc, in_maps, core_ids=list(range(N_CORES)), trace=_trace)
    out = np.stack([res.results[c]["out"] for c in range(N_CORES)], axis=0)
    if _trace:
        _cached["last_results"] = res
    return out.astype(np.float32)


if __name__ == "__main__":
    rng = np.random.default_rng(0)
    ins = {
        "q": rng.standard_normal((B, L, D)).astype(np.float32),
        "k": rng.standard_normal((B, L, D)).astype(np.float32),
        "v": rng.standard_normal((B, L, D)).astype(np.float32),
        "Wq": (rng.standard_normal((D, D)) * 0.02).astype(np.float32),
        "bq": (rng.standard_normal(D) * 0.02).astype(np.float32),
        "Wk": (rng.standard_normal((D, D)) * 0.02).astype(np.float32),
        "bk": (rng.standard_normal(D) * 0.02).astype(np.float32),
        "Wv": (rng.standard_normal((D, D)) * 0.02).astype(np.float32),
        "bv": (rng.standard_normal(D) * 0.02).astype(np.float32),
    }
    out = kernel(**ins)
    print("out", out.shape, out.dtype)


# revision 10
# speedup vs baseline: 432.5189x; 432.5189x over previous
"""Trainium2 Bass kernel for BottleNeck attention (8 NeuronCores).

Reference computation (jax, fp32):
    qp = q @ Wq.T + bq          [B=8, L=4096, D=1024]
    kp = k @ Wk.T + bk
    vp = v @ Wv.T + bv
    score = qp[:, :256] @ kp.T / sqrt(D)        [B, 256, L]
    attn  = softmax(score, axis=0)              (softmax over the BATCH axis!)
    out   = attn @ vp                           [B, 256, D]

Strategy:
  * Data-parallel over batch: core c owns batch b=c.
  * Algebraic reassociation avoids projecting full-length k/v:
        qp_T = Wq.T.T @ q.T                     [D, Q]     (per batch)
        qk_T = Wk.T @ qp_T                      [D, Q]
        score_T = k.T.T @ qk_T (+ bias row)     [L, Q]
        E = exp(score_T / 32)
        denom = AllReduce_batch(E)              (axis-0 softmax denominator)
        attn_T = E / denom                      [L, Q]
        av_T = v.T-chunks @ attn_T              [D, Q]
        out = av_T.T @ Wv.T + rowsum(attn) * bv [Q, D]
    This cuts FLOPs ~3x vs projecting kp/vp at full length.
  * Host pre-transposes (k.T, q.T, Wq.T, Wv.T) so no transposes on device.
  * Compute dtype float32r: fp32 bit layout, PE rounds to ~tf32; full PE rate
    at N>=256.

All matmuls contract over the SBUF partition dim; every operand arrives in
its natural (host pre-transposed) layout.
"""

import sys
from contextlib import ExitStack

sys.path.insert(0, "/opt/trn_rl_repo")

import numpy as np

import concourse.bass as bass
import concourse.mybir as mybir
import concourse.tile as tile
from concourse import bacc, bass_utils

B = 8
L = 4096
D = 1024
Q = 256  # bottleneck
N_CORES = 8
P = 128
DC = D // P  # 8 d-chunks
EC = D // P  # 8 e-chunks
LC = L // P  # 32 l-chunks
SCALE = 1.0 / 32.0  # 1/sqrt(1024)

# compute dtype for matmul operands ("bf16" | "fp32r" | "fp32")
COMPUTE = "bf16"
# number of AllReduce chunks (pipeline the softmax-denominator reduction)
N_AR = 2

_CDT = {
    "bf16": mybir.dt.bfloat16,
    "fp32r": mybir.dt.float32r,  # fp32 bits; PE rounds internally (~tf32)
    "fp32": mybir.dt.float32,
}

_cached = {}


def _np_cdt():
    if COMPUTE == "bf16":
        import ml_dtypes

        return np.dtype(ml_dtypes.bfloat16)
    return np.dtype(np.float32)


def build_kernel():
    CDT = _CDT[COMPUTE]
    F32 = mybir.dt.float32

    nc = bacc.Bacc("TRN2", target_bir_lowering=False, debug=False,
                   num_devices=N_CORES)

    # ---- per-core external inputs (host pre-transposed / pre-cast) ----
    kT = nc.dram_tensor("kT", [D, L], CDT, kind="ExternalInput")       # k[b].T
    v_in = nc.dram_tensor("v_in", [L, D], CDT, kind="ExternalInput")   # v[b]
    qT = nc.dram_tensor("qT", [D, Q], CDT, kind="ExternalInput")       # q[b,:Q].T
    wqT = nc.dram_tensor("wqT", [D, D], CDT, kind="ExternalInput")     # Wq.T
    wk = nc.dram_tensor("wk", [D, D], CDT, kind="ExternalInput")       # Wk
    wvT = nc.dram_tensor("wvT", [D, D], CDT, kind="ExternalInput")     # Wv.T
    bq_in = nc.dram_tensor("bq_in", [1, D], CDT, kind="ExternalInput")
    bk_in = nc.dram_tensor("bk_in", [P, EC], CDT, kind="ExternalInput")  # bk.reshape(EC,P).T
    bv_in = nc.dram_tensor("bv_in", [1, D], CDT, kind="ExternalInput")
    ones_r_in = nc.dram_tensor("ones_r", [1, Q], CDT, kind="ExternalInput")
    ones_c_in = nc.dram_tensor("ones_c", [P, 1], CDT, kind="ExternalInput")
    out_ext = nc.dram_tensor("out", [Q, D], F32, kind="ExternalOutput")

    # DRAM views with the partition-chunk structure we DMA through
    kT_v = kT.rearrange("(c p) l -> p c l", p=P)        # [128, 8, 4096]
    wqT_v = wqT.rearrange("(c p) e -> p c e", p=P)      # [128, 8, 1024]
    wk_v = wk.rearrange("(c p) d -> p c d", p=P)
    wvT_v = wvT.rearrange("(c p) e -> p c e", p=P)
    qT_v = qT.rearrange("(c p) q -> p c q", p=P)        # [128, 8, 256]
    v_v = v_in.rearrange("(c p) d -> c p d", p=P)       # [32, 128, 1024]
    out_v = out_ext.rearrange("(m p) e -> p m e", p=P)  # [128, 2, 1024]

    with tile.TileContext(nc) as tc, ExitStack() as top:
        consts = top.enter_context(tc.tile_pool(name="consts", bufs=1))
        qstate = top.enter_context(tc.tile_pool(name="qstate", bufs=1))
        dram = top.enter_context(tc.tile_pool(name="dram", bufs=1, space="DRAM"))

        # ---------------- constants ----------------
        ones_row = consts.tile([1, Q], CDT)       # [1, 256] of 1.0
        ones_col = consts.tile([P, 1], CDT)       # [128, 1] of 1.0
        nc.sync.dma_start(out=ones_row, in_=ones_r_in[:, :])
        nc.sync.dma_start(out=ones_col, in_=ones_c_in[:, :])
        bq_sb = consts.tile([1, D], CDT)
        bk_sb = consts.tile([P, EC], CDT)
        bv_sb = consts.tile([1, D], CDT)
        nc.sync.dma_start(out=bq_sb, in_=bq_in[:, :])
        nc.sync.dma_start(out=bk_sb, in_=bk_in[:, :])
        nc.sync.dma_start(out=bv_sb, in_=bv_in[:, :])

        qpT_sb = qstate.tile([P, EC, Q], CDT)
        qkT_sb = qstate.tile([P, DC, Q], CDT)
        qkb_sb = qstate.tile([1, Q], CDT)
        avT_sb = qstate.tile([P, DC, Q], CDT)
        rs_sb = qstate.tile([1, Q], CDT)

        ps4_ctx = ExitStack()
        ps4 = ps4_ctx.enter_context(
            tc.tile_pool(name="ps4", bufs=4, space="PSUM"))

        # ================ phases A+B: q-side projections ================
        with tc.tile_pool(name="wab", bufs=1) as wab:
            wqT_sb = wab.tile([P, EC, D], CDT)
            wk_sb = wab.tile([P, EC, D], CDT)
            qT_sb = wab.tile([P, DC, Q], CDT)
            nc.sync.dma_start(out=wqT_sb, in_=wqT_v)
            nc.sync.dma_start(out=wk_sb, in_=wk_v)
            nc.sync.dma_start(out=qT_sb, in_=qT_v)

            # qp_T[e,q] = sum_d WqT[d, e-chunk].T @ qT[d, q]  (+ bq x ones)
            for ec in range(EC):
                ps_qp = ps4.tile([P, Q], F32, tag="ps")
                for dc in range(DC):
                    nc.tensor.matmul(
                        ps_qp,
                        wqT_sb[:, dc, ec * P:(ec + 1) * P],
                        qT_sb[:, dc, :],
                        start=(dc == 0), stop=False,
                    )
                nc.tensor.matmul(
                    ps_qp, bq_sb[:, ec * P:(ec + 1) * P], ones_row,
                    start=False, stop=True,
                )
                nc.scalar.copy(qpT_sb[:, ec, :], ps_qp)

            # qk_T[d,q] = sum_e Wk[e, d-chunk].T @ qp_T[e, q]
            for dc in range(DC):
                ps_qk = ps4.tile([P, Q], F32, tag="ps")
                for ec in range(EC):
                    nc.tensor.matmul(
                        ps_qk,
                        wk_sb[:, ec, dc * P:(dc + 1) * P],
                        qpT_sb[:, ec, :],
                        start=(ec == 0), stop=(ec == EC - 1),
                    )
                nc.scalar.copy(qkT_sb[:, dc, :], ps_qk)
            # score bias row: qkb[q] = sum_e bk[e] * qp_T[e, q]
            ps_qkb = ps4.tile([1, Q], F32, tag="ps")
            for ec in range(EC):
                nc.tensor.matmul(
                    ps_qkb, bk_sb[:, ec:ec + 1], qpT_sb[:, ec, :],
                    start=(ec == 0), stop=(ec == EC - 1),
                )
            nc.scalar.copy(qkb_sb, ps_qkb)

        # ================ phase C: score_T -> E -> DRAM ================
        bigctx = ExitStack()
        bigbuf = bigctx.enter_context(tc.tile_pool(name="bigbuf", bufs=1))
        wvp = bigctx.enter_context(tc.tile_pool(name="wvp", bufs=1))
        ART = mybir.dt.float16  # AllReduce payload dtype (E fits fp16 range)
        E_sb = bigbuf.tile([P, LC * Q], ART)          # [128, 8192]
        wvT_sb = wvp.tile([P, DC, D], CDT)
        nc.sync.dma_start(out=wvT_sb, in_=wvT_v)      # overlaps C on DMA
        AR_COLS = LC * Q // N_AR
        E_drams = [dram.tile([P, AR_COLS], ART, name=f"E_dram_{i}")
                   for i in range(N_AR)]
        denom_drams = [dram.tile([P, AR_COLS], ART, addr_space="Shared",
                                 name=f"denom_dram_{i}") for i in range(N_AR)]

        SLAB = 4  # l-chunks per kT slab (512 l positions)
        with tc.tile_pool(name="kslabs", bufs=2) as kslabs:
            for sl in range(LC // SLAB):
                kT_t = kslabs.tile([P, DC, SLAB * P], CDT, tag="kT")
                nc.sync.dma_start(
                    out=kT_t,
                    in_=kT_v[:, :, sl * SLAB * P:(sl + 1) * SLAB * P])
                for s in range(SLAB):
                    lc = sl * SLAB + s
                    ps_s = ps4.tile([P, Q], F32, tag="ps")
                    for dc in range(DC):
                        nc.tensor.matmul(
                            ps_s,
                            kT_t[:, dc, s * P:(s + 1) * P],
                            qkT_sb[:, dc, :],
                            start=(dc == 0), stop=False,
                        )
                    nc.tensor.matmul(
                        ps_s, ones_row[:, :P], qkb_sb,
                        start=False, stop=True,
                    )
                    nc.scalar.activation(
                        out=E_sb[:, lc * Q:(lc + 1) * Q], in_=ps_s,
                        func=mybir.ActivationFunctionType.Exp, scale=SCALE,
                    )
                    ar_i = (lc * Q) // AR_COLS
                    off = lc * Q - ar_i * AR_COLS
                    nc.sync.dma_start(
                        out=E_drams[ar_i][:, off:off + Q],
                        in_=E_sb[:, lc * Q:(lc + 1) * Q],
                    )
                    # kick off this chunk's AllReduce as soon as its last
                    # E slice is stored (overlaps the rest of phase C)
                    if lc * Q + Q == (ar_i + 1) * AR_COLS:
                        nc.gpsimd.collective_compute(
                            "AllReduce", mybir.AluOpType.add,
                            replica_groups=[list(range(N_CORES))],
                            ins=[E_drams[ar_i].opt()],
                            outs=[denom_drams[ar_i].opt()],
                        )
        ps4_ctx.close()

        # ================ phase E: attn = E / denom =====================
        attnp = bigctx.enter_context(tc.tile_pool(name="attnp", bufs=1))
        rscr = bigctx.enter_context(tc.tile_pool(name="rscr", bufs=2))
        denom_sb = attnp.tile([P, LC * Q], ART)
        attn_sb = attnp.tile([P, LC * Q], CDT)
        NCH = 8
        CH = LC * Q // NCH
        for i in range(NCH):
            sli = slice(i * CH, (i + 1) * CH)
            ar_i = (i * CH) // AR_COLS
            off = i * CH - ar_i * AR_COLS
            nc.sync.dma_start(out=denom_sb[:, sli],
                              in_=denom_drams[ar_i][:, off:off + CH])
            r32 = rscr.tile([P, CH], F32, tag="r32")
            nc.vector.reciprocal(r32, denom_sb[:, sli])
            nc.vector.tensor_tensor(attn_sb[:, sli], E_sb[:, sli], r32,
                                    op=mybir.AluOpType.mult)

        # ================ phase F: av_T[d, q] accumulation ===============
        # av_T[d,q] = sum_l v[l, d-chunk].T @ attn_T[l, q]
        with (tc.tile_pool(name="accump", bufs=1, space="PSUM") as accump,
              tc.tile_pool(name="vslabs", bufs=3) as vslabs):
            av_ps = [accump.tile([P, Q], F32, name=f"av_ps_{dc}")
                     for dc in range(DC)]
            for lc in range(LC):
                v_t = vslabs.tile([P, D], CDT, tag="vt")
                nc.sync.dma_start(out=v_t, in_=v_v[lc, :, :])
                at = attn_sb[:, lc * Q:(lc + 1) * Q]
                for dc in range(DC):
                    nc.tensor.matmul(
                        av_ps[dc], v_t[:, dc * P:(dc + 1) * P], at,
                        start=(lc == 0), stop=(lc == LC - 1),
                    )
            for dc in range(DC):
                nc.scalar.copy(avT_sb[:, dc, :], av_ps[dc])

        # ================ phase G: rowsum + out projection ===============
        with (tc.tile_pool(name="outp", bufs=2, space="PSUM") as outp,
              tc.tile_pool(name="rsp", bufs=1, space="PSUM") as rsp,
              tc.tile_pool(name="outsb", bufs=2) as outsb):
            # rowsum(attn)[q] via ones-column matmul
            rs_ps = rsp.tile([1, Q], F32)
            for lc in range(LC):
                nc.tensor.matmul(
                    rs_ps, ones_col, attn_sb[:, lc * Q:(lc + 1) * Q],
                    start=(lc == 0), stop=(lc == LC - 1),
                )
            nc.scalar.copy(rs_sb, rs_ps)

            # out[q,e] = sum_d av_T[d, q-chunk].T @ WvT[d, e] + rs * bv
            NB = D // 512
            for qm in range(Q // P):
                for eb in range(NB):
                    ps_o = outp.tile([P, 512], F32, tag="ps_out")
                    for dc in range(DC):
                        nc.tensor.matmul(
                            ps_o,
                            avT_sb[:, dc, qm * P:(qm + 1) * P],
                            wvT_sb[:, dc, eb * 512:(eb + 1) * 512],
                            start=(dc == 0), stop=False,
                        )
                    nc.tensor.matmul(
                        ps_o,
                        rs_sb[:, qm * P:(qm + 1) * P],
                        bv_sb[:, eb * 512:(eb + 1) * 512],
                        start=False, stop=True,
                    )
                    o_sb = outsb.tile([P, 512], F32, tag="o_sb")
                    nc.vector.tensor_copy(o_sb, ps_o)
                    nc.sync.dma_start(
                        out=out_v[:, qm, eb * 512:(eb + 1) * 512], in_=o_sb)
        bigctx.close()

    nc.compile()
    return nc


def _prep_inputs(q, k, v, Wq, bq, Wk, bk, Wv, bv):
    """Shard + pre-transpose + cast on host. Returns in_maps for 8 cores."""
    cnp = _np_cdt()
    f32 = np.float32

    def c(x):
        return np.ascontiguousarray(np.asarray(x, dtype=f32), dtype=cnp)

    # shared across cores
    wqT = c(np.asarray(Wq, dtype=f32).T)
    wk_ = c(Wk)
    wvT = c(np.asarray(Wv, dtype=f32).T)
    bq_ = c(np.asarray(bq, dtype=f32).reshape(1, D))
    bk_ = c(np.asarray(bk, dtype=f32).reshape(EC, P).T)
    bv_ = c(np.asarray(bv, dtype=f32).reshape(1, D))
    ones_r = np.ones((1, Q), dtype=cnp)
    ones_c = np.ones((P, 1), dtype=cnp)

    in_maps = []
    for b in range(B):
        in_maps.append({
            "kT": c(np.asarray(k[b], dtype=f32).T),
            "v_in": c(v[b]),
            "qT": c(np.asarray(q[b, :Q], dtype=f32).T),
            "wqT": wqT,
            "wk": wk_,
            "wvT": wvT,
            "bq_in": bq_,
            "bk_in": bk_,
            "bv_in": bv_,
            "ones_r": ones_r,
            "ones_c": ones_c,
        })
    return in_maps


def kernel(q, k, v, Wq, bq, Wk, bk, Wv, bv, _trace=False):
    q = np.asarray(q)
    k = np.asarray(k)
    v = np.asarray(v)
    if "nc" not in _cached:
        _cached["nc"] = build_kernel()
    nc = _cached["nc"]
    in_maps = _prep_inputs(q, k, v, Wq, bq, Wk, bk, Wv, bv)
    res = bass_utils.run_bass_kernel_spmd(
        nc, in_maps, core_ids=list(range(N_CORES)), trace=_trace)
    out = np.stack([res.results[c]["out"] for c in range(N_CORES)], axis=0)
    if _trace:
        _cached["last_results"] = res
    return out.astype(np.float32)


if __name__ == "__main__":
    rng = np.random.default_rng(0)
    ins = {
        "q": rng.standard_normal((B, L, D)).astype(np.float32),
        "k": rng.standard_normal((B, L, D)).astype(np.float32),
        "v": rng.standard_normal((B, L, D)).astype(np.float32),
        "Wq": (rng.standard_normal((D, D)) * 0.02).astype(np.float32),
        "bq": (rng.standard_normal(D) * 0.02).astype(np.float32),
        "Wk": (rng.standard_normal((D, D)) * 0.02).astype(np.float32),
        "bk": (rng.standard_normal(D) * 0.02).astype(np.float32),
        "Wv": (rng.standard_normal((D, D)) * 0.02).astype(np.float32),
        "bv": (rng.standard_normal(D) * 0.02).astype(np.float32),
    }
    out = kernel(**ins)
    print("out", out.shape, out.dtype)


# revision 12
# speedup vs baseline: 442.4391x; 1.0229x over previous
"""Trainium2 Bass kernel for BottleNeck attention (8 NeuronCores).

Reference computation (jax, fp32):
    qp = q @ Wq.T + bq          [B=8, L=4096, D=1024]
    kp = k @ Wk.T + bk
    vp = v @ Wv.T + bv
    score = qp[:, :256] @ kp.T / sqrt(D)        [B, 256, L]
    attn  = softmax(score, axis=0)              (softmax over the BATCH axis!)
    out   = attn @ vp                           [B, 256, D]

Strategy:
  * Data-parallel over batch: core c owns batch b=c.
  * Algebraic reassociation avoids projecting full-length k/v:
        qp_T = Wq.T.T @ q.T                     [D, Q]     (per batch)
        qk_T = Wk.T @ qp_T                      [D, Q]
        score_T = k.T.T @ qk_T (+ bias row)     [L, Q]
        E = exp(score_T / 32)
        denom = AllReduce_batch(E)              (axis-0 softmax denominator)
        attn_T = E / denom                      [L, Q]
        av_T = v.T-chunks @ attn_T              [D, Q]
        out = av_T.T @ Wv.T + rowsum(attn) * bv [Q, D]
    This cuts FLOPs ~3x vs projecting kp/vp at full length.
  * Host pre-transposes (k.T, q.T, Wq.T, Wv.T) so no transposes on device.
  * Compute dtype float32r: fp32 bit layout, PE rounds to ~tf32; full PE rate
    at N>=256.

All matmuls contract over the SBUF partition dim; every operand arrives in
its natural (host pre-transposed) layout.
"""

import sys
from contextlib import ExitStack

sys.path.insert(0, "/opt/trn_rl_repo")

import numpy as np

import concourse.bass as bass
import concourse.mybir as mybir
import concourse.tile as tile
from concourse import bacc, bass_utils

B = 8
L = 4096
D = 1024
Q = 256  # bottleneck
N_CORES = 8
P = 128
DC = D // P  # 8 d-chunks
EC = D // P  # 8 e-chunks
LC = L // P  # 32 l-chunks
SCALE = 1.0 / 32.0  # 1/sqrt(1024)

# compute dtype for matmul operands ("bf16" | "fp32r" | "fp32")
COMPUTE = "bf16"
# number of AllReduce chunks (pipeline the softmax-denominator reduction)
N_AR = 4

_CDT = {
    "bf16": mybir.dt.bfloat16,
    "fp32r": mybir.dt.float32r,  # fp32 bits; PE rounds internally (~tf32)
    "fp32": mybir.dt.float32,
}

_cached = {}


def _np_cdt():
    if COMPUTE == "bf16":
        import ml_dtypes

        return np.dtype(ml_dtypes.bfloat16)
    return np.dtype(np.float32)


def build_kernel():
    CDT = _CDT[COMPUTE]
    F32 = mybir.dt.float32

    nc = bacc.Bacc("TRN2", target_bir_lowering=False, debug=False,
                   num_devices=N_CORES)

    # ---- per-core external inputs (host pre-transposed / pre-cast) ----
    kT = nc.dram_tensor("kT", [D, L], CDT, kind="ExternalInput")       # k[b].T
    v_in = nc.dram_tensor("v_in", [L, D], CDT, kind="ExternalInput")   # v[b]
    qT = nc.dram_tensor("qT", [D, Q], CDT, kind="ExternalInput")       # q[b,:Q].T
    wqT = nc.dram_tensor("wqT", [D, D], CDT, kind="ExternalInput")     # Wq.T
    wk = nc.dram_tensor("wk", [D, D], CDT, kind="ExternalInput")       # Wk
    wvT = nc.dram_tensor("wvT", [D, D], CDT, kind="ExternalInput")     # Wv.T
    bq_in = nc.dram_tensor("bq_in", [1, D], CDT, kind="ExternalInput")
    bk_in = nc.dram_tensor("bk_in", [P, EC], CDT, kind="ExternalInput")  # bk.reshape(EC,P).T
    bv_in = nc.dram_tensor("bv_in", [1, D], CDT, kind="ExternalInput")
    ones_r_in = nc.dram_tensor("ones_r", [1, Q], CDT, kind="ExternalInput")
    ones_c_in = nc.dram_tensor("ones_c", [P, 1], CDT, kind="ExternalInput")
    out_ext = nc.dram_tensor("out", [Q, D], F32, kind="ExternalOutput")

    # DRAM views with the partition-chunk structure we DMA through
    kT_v = kT.rearrange("(c p) l -> p c l", p=P)        # [128, 8, 4096]
    wqT_v = wqT.rearrange("(c p) e -> p c e", p=P)      # [128, 8, 1024]
    wk_v = wk.rearrange("(c p) d -> p c d", p=P)
    wvT_v = wvT.rearrange("(c p) e -> p c e", p=P)
    qT_v = qT.rearrange("(c p) q -> p c q", p=P)        # [128, 8, 256]
    v_v = v_in.rearrange("(c p) d -> c p d", p=P)       # [32, 128, 1024]
    out_v = out_ext.rearrange("(m p) e -> p m e", p=P)  # [128, 2, 1024]

    with tile.TileContext(nc) as tc, ExitStack() as top:
        consts = top.enter_context(tc.tile_pool(name="consts", bufs=1))
        qstate = top.enter_context(tc.tile_pool(name="qstate", bufs=1))
        dram = top.enter_context(tc.tile_pool(name="dram", bufs=1, space="DRAM"))

        # ---------------- constants ----------------
        ones_row = consts.tile([1, Q], CDT)       # [1, 256] of 1.0
        ones_col = consts.tile([P, 1], CDT)       # [128, 1] of 1.0
        nc.sync.dma_start(out=ones_row, in_=ones_r_in[:, :])
        nc.sync.dma_start(out=ones_col, in_=ones_c_in[:, :])
        bq_sb = consts.tile([1, D], CDT)
        bk_sb = consts.tile([P, EC], CDT)
        bv_sb = consts.tile([1, D], CDT)
        nc.sync.dma_start(out=bq_sb, in_=bq_in[:, :])
        nc.sync.dma_start(out=bk_sb, in_=bk_in[:, :])
        nc.sync.dma_start(out=bv_sb, in_=bv_in[:, :])

        qpT_sb = qstate.tile([P, EC, Q], CDT)
        qkT_sb = qstate.tile([P, DC, Q], CDT)
        qkb_sb = qstate.tile([1, Q], CDT)
        avT_sb = qstate.tile([P, DC, Q], CDT)
        rs_sb = qstate.tile([1, Q], CDT)

        ps4_ctx = ExitStack()
        ps4 = ps4_ctx.enter_context(
            tc.tile_pool(name="ps4", bufs=4, space="PSUM"))

        # ================ phases A+B: q-side projections ================
        with tc.tile_pool(name="wab", bufs=1) as wab:
            wqT_sb = wab.tile([P, EC, D], CDT)
            wk_sb = wab.tile([P, EC, D], CDT)
            qT_sb = wab.tile([P, DC, Q], CDT)
            nc.sync.dma_start(out=wqT_sb, in_=wqT_v)
            nc.sync.dma_start(out=wk_sb, in_=wk_v)
            nc.sync.dma_start(out=qT_sb, in_=qT_v)

            # qp_T[e,q] = sum_d WqT[d, e-chunk].T @ qT[d, q]  (+ bq x ones)
            for ec in range(EC):
                ps_qp = ps4.tile([P, Q], F32, tag="ps")
                for dc in range(DC):
                    nc.tensor.matmul(
                        ps_qp,
                        wqT_sb[:, dc, ec * P:(ec + 1) * P],
                        qT_sb[:, dc, :],
                        start=(dc == 0), stop=False,
                    )
                nc.tensor.matmul(
                    ps_qp, bq_sb[:, ec * P:(ec + 1) * P], ones_row,
                    start=False, stop=True,
                )
                nc.scalar.copy(qpT_sb[:, ec, :], ps_qp)

            # qk_T[d,q] = sum_e Wk[e, d-chunk].T @ qp_T[e, q]
            for dc in range(DC):
                ps_qk = ps4.tile([P, Q], F32, tag="ps")
                for ec in range(EC):
                    nc.tensor.matmul(
                        ps_qk,
                        wk_sb[:, ec, dc * P:(dc + 1) * P],
                        qpT_sb[:, ec, :],
                        start=(ec == 0), stop=(ec == EC - 1),
                    )
                nc.scalar.copy(qkT_sb[:, dc, :], ps_qk)
            # score bias row: qkb[q] = sum_e bk[e] * qp_T[e, q]
            ps_qkb = ps4.tile([1, Q], F32, tag="ps")
            for ec in range(EC):
                nc.tensor.matmul(
                    ps_qkb, bk_sb[:, ec:ec + 1], qpT_sb[:, ec, :],
                    start=(ec == 0), stop=(ec == EC - 1),
                )
            nc.scalar.copy(qkb_sb, ps_qkb)

        # ================ phase C: score_T -> E -> DRAM ================
        bigctx = ExitStack()
        bigbuf = bigctx.enter_context(tc.tile_pool(name="bigbuf", bufs=1))
        wvp = bigctx.enter_context(tc.tile_pool(name="wvp", bufs=1))
        ART = mybir.dt.float16  # AllReduce payload dtype (E fits fp16 range)
        E_sb = bigbuf.tile([P, LC * Q], ART)          # [128, 8192]
        wvT_sb = wvp.tile([P, DC, D], CDT)
        nc.sync.dma_start(out=wvT_sb, in_=wvT_v)      # overlaps C on DMA
        AR_COLS = LC * Q // N_AR
        E_drams = [dram.tile([P, AR_COLS], ART, name=f"E_dram_{i}")
                   for i in range(N_AR)]
        denom_drams = [dram.tile([P, AR_COLS], ART, addr_space="Shared",
                                 name=f"denom_dram_{i}") for i in range(N_AR)]

        SLAB = 4  # l-chunks per kT slab (512 l positions)
        with tc.tile_pool(name="kslabs", bufs=2) as kslabs:
            for sl in range(LC // SLAB):
                kT_t = kslabs.tile([P, DC, SLAB * P], CDT, tag="kT")
                nc.sync.dma_start(
                    out=kT_t,
                    in_=kT_v[:, :, sl * SLAB * P:(sl + 1) * SLAB * P])
                for s in range(SLAB):
                    lc = sl * SLAB + s
                    ps_s = ps4.tile([P, Q], F32, tag="ps")
                    for dc in range(DC):
                        nc.tensor.matmul(
                            ps_s,
                            kT_t[:, dc, s * P:(s + 1) * P],
                            qkT_sb[:, dc, :],
                            start=(dc == 0), stop=False,
                        )
                    nc.tensor.matmul(
                        ps_s, ones_row[:, :P], qkb_sb,
                        start=False, stop=True,
                    )
                    nc.scalar.activation(
                        out=E_sb[:, lc * Q:(lc + 1) * Q], in_=ps_s,
                        func=mybir.ActivationFunctionType.Exp, scale=SCALE,
                    )
                    ar_i = (lc * Q) // AR_COLS
                    off = lc * Q - ar_i * AR_COLS
                    nc.sync.dma_start(
                        out=E_drams[ar_i][:, off:off + Q],
                        in_=E_sb[:, lc * Q:(lc + 1) * Q],
                    )
                    # kick off this chunk's AllReduce as soon as its last
                    # E slice is stored (overlaps the rest of phase C)
                    if lc * Q + Q == (ar_i + 1) * AR_COLS:
                        nc.gpsimd.collective_compute(
                            "AllReduce", mybir.AluOpType.add,
                            replica_groups=[list(range(N_CORES))],
                            ins=[E_drams[ar_i].opt()],
                            outs=[denom_drams[ar_i].opt()],
                        )
        ps4_ctx.close()

        # ====== phases E+F interleaved per AR chunk: attn then av_T ======
        # attn = E * recip(denom) (recip on ScalarE: one exp->recip table
        # switch; DVE only does the multiply). As soon as one AR chunk's
        # denominator lands, its attn chunk is formed and its av_T matmuls
        # run -- overlapping the remaining AllReduce chunks.
        attnp = bigctx.enter_context(tc.tile_pool(name="attnp", bufs=1))
        rscr = bigctx.enter_context(tc.tile_pool(name="rscr", bufs=2))
        denom_sb = attnp.tile([P, LC * Q], ART)
        attn_sb = attnp.tile([P, LC * Q], CDT)
        CH = 1024  # DVE/ACT chunk (4 l-chunks)
        LC_PER_AR = LC // N_AR
        with (tc.tile_pool(name="accump", bufs=1, space="PSUM") as accump,
              tc.tile_pool(name="vslabs", bufs=3) as vslabs):
            av_ps = [accump.tile([P, Q], F32, name=f"av_ps_{dc}")
                     for dc in range(DC)]
            for ar_i in range(N_AR):
                for j in range(AR_COLS // CH):
                    off = j * CH
                    g = ar_i * AR_COLS + off
                    sli = slice(g, g + CH)
                    nc.sync.dma_start(out=denom_sb[:, sli],
                                      in_=denom_drams[ar_i][:, off:off + CH])
                    r32 = rscr.tile([P, CH], F32, tag="r32")
                    nc.scalar.copy(r32, denom_sb[:, sli])  # fp16 -> fp32
                    nc.vector.reciprocal_approx_fast(r32, r32)
                    nc.vector.tensor_tensor(attn_sb[:, sli], E_sb[:, sli],
                                            r32, op=mybir.AluOpType.mult)
                for lc in range(ar_i * LC_PER_AR, (ar_i + 1) * LC_PER_AR):
                    v_t = vslabs.tile([P, D], CDT, tag="vt")
                    nc.sync.dma_start(out=v_t, in_=v_v[lc, :, :])
                    at = attn_sb[:, lc * Q:(lc + 1) * Q]
                    for dc in range(DC):
                        nc.tensor.matmul(
                            av_ps[dc], v_t[:, dc * P:(dc + 1) * P], at,
                            start=(lc == 0), stop=(lc == LC - 1),
                        )
            for dc in range(DC):
                nc.scalar.copy(avT_sb[:, dc, :], av_ps[dc])

        # ================ phase G: rowsum + out projection ===============
        with (tc.tile_pool(name="outp", bufs=2, space="PSUM") as outp,
              tc.tile_pool(name="rsp", bufs=1, space="PSUM") as rsp,
              tc.tile_pool(name="outsb", bufs=2) as outsb):
            # rowsum(attn)[q] via ones-column matmul
            rs_ps = rsp.tile([1, Q], F32)
            for lc in range(LC):
                nc.tensor.matmul(
                    rs_ps, ones_col, attn_sb[:, lc * Q:(lc + 1) * Q],
                    start=(lc == 0), stop=(lc == LC - 1),
                )
            nc.scalar.copy(rs_sb, rs_ps)

            # out[q,e] = sum_d av_T[d, q-chunk].T @ WvT[d, e] + rs * bv
            NB = D // 512
            for qm in range(Q // P):
                for eb in range(NB):
                    ps_o = outp.tile([P, 512], F32, tag="ps_out")
                    for dc in range(DC):
                        nc.tensor.matmul(
                            ps_o,
                            avT_sb[:, dc, qm * P:(qm + 1) * P],
                            wvT_sb[:, dc, eb * 512:(eb + 1) * 512],
                            start=(dc == 0), stop=False,
                        )
                    nc.tensor.matmul(
                        ps_o,
                        rs_sb[:, qm * P:(qm + 1) * P],
                        bv_sb[:, eb * 512:(eb + 1) * 512],
                        start=False, stop=True,
                    )
                    o_sb = outsb.tile([P, 512], F32, tag="o_sb")
                    nc.vector.tensor_copy(o_sb, ps_o)
                    nc.sync.dma_start(
                        out=out_v[:, qm, eb * 512:(eb + 1) * 512], in_=o_sb)
        bigctx.close()

    nc.compile()
    return nc


def _prep_inputs(q, k, v, Wq, bq, Wk, bk, Wv, bv):
    """Shard + pre-transpose + cast on host. Returns in_maps for 8 cores."""
    cnp = _np_cdt()
    f32 = np.float32

    def c(x):
        return np.ascontiguousarray(np.asarray(x, dtype=f32), dtype=cnp)

    # shared across cores
    wqT = c(np.asarray(Wq, dtype=f32).T)
    wk_ = c(Wk)
    wvT = c(np.asarray(Wv, dtype=f32).T)
    bq_ = c(np.asarray(bq, dtype=f32).reshape(1, D))
    bk_ = c(np.asarray(bk, dtype=f32).reshape(EC, P).T)
    bv_ = c(np.asarray(bv, dtype=f32).reshape(1, D))
    ones_r = np.ones((1, Q), dtype=cnp)
    ones_c = np.ones((P, 1), dtype=cnp)

    in_maps = []
    for b in range(B):
        in_maps.append({
            "kT": c(np.asarray(k[b], dtype=f32).T),
            "v_in": c(v[b]),
            "qT": c(np.asarray(q[b, :Q], dtype=f32).T),
            "wqT": wqT,
            "wk": wk_,
            "wvT": wvT,
            "bq_in": bq_,
            "bk_in": bk_,
            "bv_in": bv_,
            "ones_r": ones_r,
            "ones_c": ones_c,
        })
    return in_maps


def kernel(q, k, v, Wq, bq, Wk, bk, Wv, bv, _trace=False):
    q = np.asarray(q)
    k = np.asarray(k)
    v = np.asarray(v)
    if "nc" not in _cached:
        _cached["nc"] = build_kernel()
    nc = _cached["nc"]
    in_maps = _prep_inputs(q, k, v, Wq, bq, Wk, bk, Wv, bv)
    res = bass_utils.run_bass_kernel_spmd(
        nc, in_maps, core_ids=list(range(N_CORES)), trace=_trace)
    out = np.stack([res.results[c]["out"] for c in range(N_CORES)], axis=0)
    if _trace:
        _cached["last_results"] = res
    return out.astype(np.float32)


if __name__ == "__main__":
    rng = np.random.default_rng(0)
    ins = {
        "q": rng.standard_normal((B, L, D)).astype(np.float32),
        "k": rng.standard_normal((B, L, D)).astype(np.float32),
        "v": rng.standard_normal((B, L, D)).astype(np.float32),
        "Wq": (rng.standard_normal((D, D)) * 0.02).astype(np.float32),
        "bq": (rng.standard_normal(D) * 0.02).astype(np.float32),
        "Wk": (rng.standard_normal((D, D)) * 0.02).astype(np.float32),
        "bk": (rng.standard_normal(D) * 0.02).astype(np.float32),
        "Wv": (rng.standard_normal((D, D)) * 0.02).astype(np.float32),
        "bv": (rng.standard_normal(D) * 0.02).astype(np.float32),
    }
    out = kernel(**ins)
    print("out", out.shape, out.dtype)


# revision 13
# speedup vs baseline: 462.9874x; 1.0464x over previous
"""Trainium2 Bass kernel for BottleNeck attention (8 NeuronCores).

Reference computation (jax, fp32):
    qp = q @ Wq.T + bq          [B=8, L=4096, D=1024]
    kp = k @ Wk.T + bk
    vp = v @ Wv.T + bv
    score = qp[:, :256] @ kp.T / sqrt(D)        [B, 256, L]
    attn  = softmax(score, axis=0)              (softmax over the BATCH axis!)
    out   = attn @ vp                           [B, 256, D]

Strategy:
  * Data-parallel over batch: core c owns batch b=c.
  * Algebraic reassociation avoids projecting full-length k/v:
        qp_T = Wq.T.T @ q.T                     [D, Q]     (per batch)
        qk_T = Wk.T @ qp_T                      [D, Q]
        score_T = k.T.T @ qk_T (+ bias row)     [L, Q]
        E = exp(score_T / 32)
        denom = AllReduce_batch(E)              (axis-0 softmax denominator)
        attn_T = E / denom                      [L, Q]
        av_T = v.T-chunks @ attn_T              [D, Q]
        out = av_T.T @ Wv.T + rowsum(attn) * bv [Q, D]
    This cuts FLOPs ~3x vs projecting kp/vp at full length.
  * Host pre-transposes (k.T, q.T, Wq.T, Wv.T) so no transposes on device.
  * Compute dtype float32r: fp32 bit layout, PE rounds to ~tf32; full PE rate
    at N>=256.

All matmuls contract over the SBUF partition dim; every operand arrives in
its natural (host pre-transposed) layout.
"""

import sys
from contextlib import ExitStack

sys.path.insert(0, "/opt/trn_rl_repo")

import numpy as np

import concourse.bass as bass
import concourse.mybir as mybir
import concourse.tile as tile
from concourse import bacc, bass_utils

B = 8
L = 4096
D = 1024
Q = 256  # bottleneck
N_CORES = 8
P = 128
DC = D // P  # 8 d-chunks
EC = D // P  # 8 e-chunks
LC = L // P  # 32 l-chunks
SCALE = 1.0 / 32.0  # 1/sqrt(1024)

# compute dtype for matmul operands ("bf16" | "fp32r" | "fp32")
COMPUTE = "bf16"
# number of AllReduce chunks (pipeline the softmax-denominator reduction)
N_AR = 4

_CDT = {
    "bf16": mybir.dt.bfloat16,
    "fp32r": mybir.dt.float32r,  # fp32 bits; PE rounds internally (~tf32)
    "fp32": mybir.dt.float32,
}

_cached = {}


def _np_cdt():
    if COMPUTE == "bf16":
        import ml_dtypes

        return np.dtype(ml_dtypes.bfloat16)
    return np.dtype(np.float32)


def build_kernel():
    CDT = _CDT[COMPUTE]
    F32 = mybir.dt.float32

    nc = bacc.Bacc("TRN2", target_bir_lowering=False, debug=False,
                   num_devices=N_CORES)

    # ---- per-core external inputs (host pre-transposed / pre-cast) ----
    kT = nc.dram_tensor("kT", [D, L], CDT, kind="ExternalInput")       # k[b].T
    v_in = nc.dram_tensor("v_in", [L, D], CDT, kind="ExternalInput")   # v[b]
    qT = nc.dram_tensor("qT", [D, Q], CDT, kind="ExternalInput")       # q[b,:Q].T
    wqT = nc.dram_tensor("wqT", [D, D], CDT, kind="ExternalInput")     # Wq.T
    wk = nc.dram_tensor("wk", [D, D], CDT, kind="ExternalInput")       # Wk
    wvT = nc.dram_tensor("wvT", [D, D], CDT, kind="ExternalInput")     # Wv.T
    bq_in = nc.dram_tensor("bq_in", [1, D], CDT, kind="ExternalInput")
    bk_in = nc.dram_tensor("bk_in", [P, EC], CDT, kind="ExternalInput")  # bk.reshape(EC,P).T
    bv_in = nc.dram_tensor("bv_in", [1, D], CDT, kind="ExternalInput")
    ones_r_in = nc.dram_tensor("ones_r", [1, Q], CDT, kind="ExternalInput")
    ones_c_in = nc.dram_tensor("ones_c", [P, 1], CDT, kind="ExternalInput")
    out_ext = nc.dram_tensor("out", [Q, D], F32, kind="ExternalOutput")

    # DRAM views with the partition-chunk structure we DMA through
    kT_v = kT.rearrange("(c p) l -> p c l", p=P)        # [128, 8, 4096]
    wqT_v = wqT.rearrange("(c p) e -> p c e", p=P)      # [128, 8, 1024]
    wk_v = wk.rearrange("(c p) d -> p c d", p=P)
    wvT_v = wvT.rearrange("(c p) e -> p c e", p=P)
    qT_v = qT.rearrange("(c p) q -> p c q", p=P)        # [128, 8, 256]
    v_v = v_in.rearrange("(c p) d -> c p d", p=P)       # [32, 128, 1024]
    out_v = out_ext.rearrange("(m p) e -> p m e", p=P)  # [128, 2, 1024]

    with tile.TileContext(nc) as tc, ExitStack() as top:
        consts = top.enter_context(tc.tile_pool(name="consts", bufs=1))
        qstate = top.enter_context(tc.tile_pool(name="qstate", bufs=1))
        dram = top.enter_context(tc.tile_pool(name="dram", bufs=1, space="DRAM"))

        # ---------------- constants ----------------
        ones_row = consts.tile([1, Q], CDT)       # [1, 256] of 1.0
        ones_col = consts.tile([P, 1], CDT)       # [128, 1] of 1.0
        nc.sync.dma_start(out=ones_row, in_=ones_r_in[:, :])
        nc.sync.dma_start(out=ones_col, in_=ones_c_in[:, :])
        bq_sb = consts.tile([1, D], CDT)
        bk_sb = consts.tile([P, EC], CDT)
        bv_sb = consts.tile([1, D], CDT)
        nc.sync.dma_start(out=bq_sb, in_=bq_in[:, :])
        nc.sync.dma_start(out=bk_sb, in_=bk_in[:, :])
        nc.sync.dma_start(out=bv_sb, in_=bv_in[:, :])

        qpT_sb = qstate.tile([P, EC, Q], CDT)
        qkT_sb = qstate.tile([P, DC, Q], CDT)
        qkb_sb = qstate.tile([1, Q], CDT)
        avT_sb = qstate.tile([P, DC, Q], CDT)
        rs_sb = qstate.tile([1, Q], CDT)

        ps4_ctx = ExitStack()
        ps4 = ps4_ctx.enter_context(
            tc.tile_pool(name="ps4", bufs=4, space="PSUM"))

        # ================ phases A+B: q-side projections ================
        with tc.tile_pool(name="wab", bufs=1) as wab:
            wqT_sb = wab.tile([P, EC, D], CDT)
            wk_sb = wab.tile([P, EC, D], CDT)
            qT_sb = wab.tile([P, DC, Q], CDT)
            # split the A-phase weight loads per d-chunk so the first
            # qp_T matmuls start after 1/8 of the DMA instead of all of it
            for dc in range(DC):
                nc.sync.dma_start(out=wqT_sb[:, dc, :], in_=wqT_v[:, dc, :])
                nc.sync.dma_start(out=qT_sb[:, dc, :], in_=qT_v[:, dc, :])
            nc.sync.dma_start(out=wk_sb, in_=wk_v)

            # qp_T[e,q] = sum_d WqT[d, e-chunk].T @ qT[d, q]  (+ bq x ones)
            for ec in range(EC):
                ps_qp = ps4.tile([P, Q], F32, tag="ps")
                for dc in range(DC):
                    nc.tensor.matmul(
                        ps_qp,
                        wqT_sb[:, dc, ec * P:(ec + 1) * P],
                        qT_sb[:, dc, :],
                        start=(dc == 0), stop=False,
                    )
                nc.tensor.matmul(
                    ps_qp, bq_sb[:, ec * P:(ec + 1) * P], ones_row,
                    start=False, stop=True,
                )
                nc.scalar.copy(qpT_sb[:, ec, :], ps_qp)

            # qk_T[d,q] = sum_e Wk[e, d-chunk].T @ qp_T[e, q]
            for dc in range(DC):
                ps_qk = ps4.tile([P, Q], F32, tag="ps")
                for ec in range(EC):
                    nc.tensor.matmul(
                        ps_qk,
                        wk_sb[:, ec, dc * P:(dc + 1) * P],
                        qpT_sb[:, ec, :],
                        start=(ec == 0), stop=(ec == EC - 1),
                    )
                nc.scalar.copy(qkT_sb[:, dc, :], ps_qk)
            # score bias row: qkb[q] = sum_e bk[e] * qp_T[e, q]
            ps_qkb = ps4.tile([1, Q], F32, tag="ps")
            for ec in range(EC):
                nc.tensor.matmul(
                    ps_qkb, bk_sb[:, ec:ec + 1], qpT_sb[:, ec, :],
                    start=(ec == 0), stop=(ec == EC - 1),
                )
            nc.scalar.copy(qkb_sb, ps_qkb)

        # ================ phase C: score_T -> E -> DRAM ================
        bigctx = ExitStack()
        bigbuf = bigctx.enter_context(tc.tile_pool(name="bigbuf", bufs=1))
        wvp = bigctx.enter_context(tc.tile_pool(name="wvp", bufs=1))
        ART = mybir.dt.float16  # AllReduce payload dtype (E fits fp16 range)
        E_sb = bigbuf.tile([P, LC * Q], ART)          # [128, 8192]
        wvT_sb = wvp.tile([P, DC, D], CDT)
        nc.sync.dma_start(out=wvT_sb, in_=wvT_v)      # overlaps C on DMA
        AR_COLS = LC * Q // N_AR
        E_drams = [dram.tile([P, AR_COLS], ART, name=f"E_dram_{i}")
                   for i in range(N_AR)]
        denom_drams = [dram.tile([P, AR_COLS], ART, addr_space="Shared",
                                 name=f"denom_dram_{i}") for i in range(N_AR)]

        SLAB = 4  # l-chunks per kT slab (512 l positions)
        with tc.tile_pool(name="kslabs", bufs=3) as kslabs:
            for sl in range(LC // SLAB):
                kT_t = kslabs.tile([P, DC, SLAB * P], CDT, tag="kT")
                nc.sync.dma_start(
                    out=kT_t,
                    in_=kT_v[:, :, sl * SLAB * P:(sl + 1) * SLAB * P])
                for s in range(SLAB):
                    lc = sl * SLAB + s
                    ps_s = ps4.tile([P, Q], F32, tag="ps")
                    for dc in range(DC):
                        nc.tensor.matmul(
                            ps_s,
                            kT_t[:, dc, s * P:(s + 1) * P],
                            qkT_sb[:, dc, :],
                            start=(dc == 0), stop=False,
                        )
                    nc.tensor.matmul(
                        ps_s, ones_row[:, :P], qkb_sb,
                        start=False, stop=True,
                    )
                    nc.scalar.activation(
                        out=E_sb[:, lc * Q:(lc + 1) * Q], in_=ps_s,
                        func=mybir.ActivationFunctionType.Exp, scale=SCALE,
                    )
                # one batched E store per slab (4 l-chunks, 2KB lines)
                g0 = sl * SLAB * Q
                ar_i = g0 // AR_COLS
                off = g0 - ar_i * AR_COLS
                W = SLAB * Q
                nc.sync.dma_start(
                    out=E_drams[ar_i][:, off:off + W],
                    in_=E_sb[:, g0:g0 + W],
                )
                # kick off this chunk's AllReduce as soon as its last
                # E slice is stored (overlaps the rest of phase C)
                if g0 + W == (ar_i + 1) * AR_COLS:
                    nc.gpsimd.collective_compute(
                        "AllReduce", mybir.AluOpType.add,
                        replica_groups=[list(range(N_CORES))],
                        ins=[E_drams[ar_i].opt()],
                        outs=[denom_drams[ar_i].opt()],
                    )
        ps4_ctx.close()

        # ====== phases E+F interleaved per AR chunk: attn then av_T ======
        # attn = E * recip(denom) (recip on ScalarE: one exp->recip table
        # switch; DVE only does the multiply). As soon as one AR chunk's
        # denominator lands, its attn chunk is formed and its av_T matmuls
        # run -- overlapping the remaining AllReduce chunks.
        attnp = bigctx.enter_context(tc.tile_pool(name="attnp", bufs=1))
        rscr = bigctx.enter_context(tc.tile_pool(name="rscr", bufs=2))
        denom_sb = attnp.tile([P, LC * Q], ART)
        attn_sb = attnp.tile([P, LC * Q], CDT)
        CH = 1024  # DVE/ACT chunk (4 l-chunks)
        LC_PER_AR = LC // N_AR
        with (tc.tile_pool(name="accump", bufs=1, space="PSUM") as accump,
              tc.tile_pool(name="vslabs", bufs=3) as vslabs):
            av_ps = [accump.tile([P, Q], F32, name=f"av_ps_{dc}")
                     for dc in range(DC)]
            for ar_i in range(N_AR):
                for j in range(AR_COLS // CH):
                    off = j * CH
                    g = ar_i * AR_COLS + off
                    sli = slice(g, g + CH)
                    nc.sync.dma_start(out=denom_sb[:, sli],
                                      in_=denom_drams[ar_i][:, off:off + CH])
                    r32 = rscr.tile([P, CH], F32, tag="r32")
                    nc.scalar.copy(r32, denom_sb[:, sli])  # fp16 -> fp32
                    nc.vector.reciprocal_approx_fast(r32, r32)
                    nc.vector.tensor_tensor(attn_sb[:, sli], E_sb[:, sli],
                                            r32, op=mybir.AluOpType.mult)
                for lc in range(ar_i * LC_PER_AR, (ar_i + 1) * LC_PER_AR):
                    v_t = vslabs.tile([P, D], CDT, tag="vt")
                    nc.sync.dma_start(out=v_t, in_=v_v[lc, :, :])
                    at = attn_sb[:, lc * Q:(lc + 1) * Q]
                    for dc in range(DC):
                        nc.tensor.matmul(
                            av_ps[dc], v_t[:, dc * P:(dc + 1) * P], at,
                            start=(lc == 0), stop=(lc == LC - 1),
                        )
            for dc in range(DC):
                nc.scalar.copy(avT_sb[:, dc, :], av_ps[dc])

        # ================ phase G: rowsum + out projection ===============
        with (tc.tile_pool(name="outp", bufs=2, space="PSUM") as outp,
              tc.tile_pool(name="rsp", bufs=1, space="PSUM") as rsp,
              tc.tile_pool(name="outsb", bufs=2) as outsb):
            # rowsum(attn)[q] via ones-column matmul
            rs_ps = rsp.tile([1, Q], F32)
            for lc in range(LC):
                nc.tensor.matmul(
                    rs_ps, ones_col, attn_sb[:, lc * Q:(lc + 1) * Q],
                    start=(lc == 0), stop=(lc == LC - 1),
                )
            nc.scalar.copy(rs_sb, rs_ps)

            # out[q,e] = sum_d av_T[d, q-chunk].T @ WvT[d, e] + rs * bv
            NB = D // 512
            for qm in range(Q // P):
                for eb in range(NB):
                    ps_o = outp.tile([P, 512], F32, tag="ps_out")
                    for dc in range(DC):
                        nc.tensor.matmul(
                            ps_o,
                            avT_sb[:, dc, qm * P:(qm + 1) * P],
                            wvT_sb[:, dc, eb * 512:(eb + 1) * 512],
                            start=(dc == 0), stop=False,
                        )
                    nc.tensor.matmul(
                        ps_o,
                        rs_sb[:, qm * P:(qm + 1) * P],
                        bv_sb[:, eb * 512:(eb + 1) * 512],
                        start=False, stop=True,
                    )
                    o_sb = outsb.tile([P, 512], F32, tag="o_sb")
                    nc.vector.tensor_copy(o_sb, ps_o)
                    nc.sync.dma_start(
                        out=out_v[:, qm, eb * 512:(eb + 1) * 512], in_=o_sb)
        bigctx.close()

    nc.compile()
    return nc


def _prep_inputs(q, k, v, Wq, bq, Wk, bk, Wv, bv):
    """Shard + pre-transpose + cast on host. Returns in_maps for 8 cores."""
    cnp = _np_cdt()
    f32 = np.float32

    def c(x):
        return np.ascontiguousarray(np.asarray(x, dtype=f32), dtype=cnp)

    # shared across cores
    wqT = c(np.asarray(Wq, dtype=f32).T)
    wk_ = c(Wk)
    wvT = c(np.asarray(Wv, dtype=f32).T)
    bq_ = c(np.asarray(bq, dtype=f32).reshape(1, D))
    bk_ = c(np.asarray(bk, dtype=f32).reshape(EC, P).T)
    bv_ = c(np.asarray(bv, dtype=f32).reshape(1, D))
    ones_r = np.ones((1, Q), dtype=cnp)
    ones_c = np.ones((P, 1), dtype=cnp)

    in_maps = []
    for b in range(B):
        in_maps.append({
            "kT": c(np.asarray(k[b], dtype=f32).T),
            "v_in": c(v[b]),
            "qT": c(np.asarray(q[b, :Q], dtype=f32).T),
            "wqT": wqT,
            "wk": wk_,
            "wvT": wvT,
            "bq_in": bq_,
            "bk_in": bk_,
            "bv_in": bv_,
            "ones_r": ones_r,
            "ones_c": ones_c,
        })
    return in_maps


def kernel(q, k, v, Wq, bq, Wk, bk, Wv, bv, _trace=False):
    q = np.asarray(q)
    k = np.asarray(k)
    v = np.asarray(v)
    if "nc" not in _cached:
        _cached["nc"] = build_kernel()
    nc = _cached["nc"]
    in_maps = _prep_inputs(q, k, v, Wq, bq, Wk, bk, Wv, bv)
    res = bass_utils.run_bass_kernel_spmd(
        nc, in_maps, core_ids=list(range(N_CORES)), trace=_trace)
    out = np.stack([res.results[c]["out"] for c in range(N_CORES)], axis=0)
    if _trace:
        _cached["last_results"] = res
    return out.astype(np.float32)


if __name__ == "__main__":
    rng = np.random.default_rng(0)
    ins = {
        "q": rng.standard_normal((B, L, D)).astype(np.float32),
        "k": rng.standard_normal((B, L, D)).astype(np.float32),
        "v": rng.standard_normal((B, L, D)).astype(np.float32),
        "Wq": (rng.standard_normal((D, D)) * 0.02).astype(np.float32),
        "bq": (rng.standard_normal(D) * 0.02).astype(np.float32),
        "Wk": (rng.standard_normal((D, D)) * 0.02).astype(np.float32),
        "bk": (rng.standard_normal(D) * 0.02).astype(np.float32),
        "Wv": (rng.standard_normal((D, D)) * 0.02).astype(np.float32),
        "bv": (rng.standard_normal(D) * 0.02).astype(np.float32),
    }
    out = kernel(**ins)
    print("out", out.shape, out.dtype)
